# revision 8
# baseline (speedup 1.0000x reference)
"""DCRNN decoder (2-layer DCGRU, K=2 diffusion, 2 supports) on 8 TRN2 NeuronCores.

Strategy: data-parallel over batch (B=32 -> 4 per core), supports streamed from
HBM as bf16 S^T column-blocks, all matmul work in bf16 with fp32 PSUM
accumulation.  Chebyshev "2*S@x1 - x0" terms and biases are folded into
host-side weight adjustments so the whole gconv is pure matmul accumulation:

  gconv(x0) = x0@(WI - W2f - W2b) + sum_s [ (Ss@x0)@W1s + (Ss@(Ss@x0))@(2*W2s) ] + b

S-applications run with the S^T tile stationary and activations as the moving
operand (node-major PSUM output).  Projections contract the feature dim, so
activation tiles are PE-transposed per (m-tile, batch, feature-group) on the
fly.  For D=128 gconvs, features are handled as two 64-wide groups and the
weights are split into 64-row halves host-side (matmul operands must share a
base partition).  For the layer-1 candidate gconv (D=128 -> out=64) the
projection-first form halves the second diffusion matmul width.
"""

import sys

import numpy as np

sys.path.insert(0, "/opt/trn_rl_repo")

import ml_dtypes

import concourse.bacc as bacc
import concourse.mybir as mybir
import concourse.tile as tile
from concourse import masks

bf16 = ml_dtypes.bfloat16
DT = mybir.dt.bfloat16
F32 = mybir.dt.float32
AF = mybir.ActivationFunctionType

B = 32
U = 64
NCORES = 8
BL = B // NCORES          # batch per core
D0 = U + 1                # layer-0 gconv input width
D1 = 2 * U                # layer-1 gconv input width
WNAMES = ("wI", "w1f", "w1b", "w2f", "w2b")
GCONVS = {
    "g0": dict(D=D0, OW=2 * U),
    "c0": dict(D=D0, OW=U),
    "g1": dict(D=D1, OW=2 * U),
    "c1": dict(D=D1, OW=U),
}


def bf(x):
    return np.ascontiguousarray(np.asarray(x, dtype=np.float32)).astype(bf16)


# ---------------------------------------------------------------------------
# device program
# ---------------------------------------------------------------------------

def build_program(n_nodes):
    NT = n_nodes // 128
    nc = bacc.Bacc("TRN2", target_bir_lowering=False, debug=False,
                   num_devices=NCORES)

    dram = {}

    def din(name, shape, dt=DT):
        dram[name] = nc.dram_tensor(name, shape, dt, kind="ExternalInput")
        return dram[name]

    def dout(name, shape, dt=DT):
        dram[name] = nc.dram_tensor(name, shape, dt, kind="ExternalOutput")
        return dram[name]

    STf_d = din("STf", (n_nodes, n_nodes))
    STb_d = din("STb", (n_nodes, n_nodes))
    din("x0g0", (n_nodes, BL * D0))
    din("x0c0", (n_nodes, BL * D0))
    din("h0", (n_nodes, BL * U))
    din("h1", (n_nodes, BL * U))
    for g, cfg in GCONVS.items():
        if cfg["D"] == D0:
            for w in WNAMES:
                din(f"{g}_{w}", (D0, cfg["OW"]))
        else:
            for w in WNAMES:
                din(f"{g}_{w}a", (U, cfg["OW"]))
                din(f"{g}_{w}b", (U, cfg["OW"]))
        din(f"{g}_bias", (1, BL * cfg["OW"]))
    Wp_d = din("Wp", (U, 1))
    bp_d = din("bp", (1, BL))

    pred_d = dout("pred", (n_nodes, BL), F32)
    h0o_d = dout("h0o", (n_nodes, BL * U))
    h1o_d = dout("h1o", (n_nodes, BL * U))

    def dram_tiled(d):
        # [N, C] dram -> [128, NT, C] node-major tile view
        return d.rearrange("(t p) c -> p t c", p=128)

    with tile.TileContext(nc) as tc:
        with (
            tc.tile_pool(name="persist", bufs=1) as pp,
            tc.tile_pool(name="wpool", bufs=1) as wp,
            tc.tile_pool(name="stream", bufs=3) as stp,
            tc.tile_pool(name="trans", bufs=4) as trp,
            tc.tile_pool(name="small", bufs=3) as smp,
            tc.tile_pool(name="pOut", bufs=2, space="PSUM") as pOut,
            tc.tile_pool(name="pAux", bufs=2, space="PSUM") as pAux,
            tc.tile_pool(name="pT", bufs=2, space="PSUM") as pT,
            tc.tile_pool(name="pS", bufs=2, space="PSUM") as pS,
        ):
            # ---- persistent SBUF tensors ----
            x0g0 = pp.tile([128, NT, BL * D0], DT, tag="x0g0")
            x0c0 = pp.tile([128, NT, BL * D0], DT, tag="x0c0")
            h0 = pp.tile([128, NT, BL * U], DT, tag="h0")
            h1 = pp.tile([128, NT, BL * U], DT, tag="h1")
            h0p = pp.tile([128, NT, BL * U], DT, tag="h0p")
            u_sb = pp.tile([128, NT, BL * U], DT, tag="u")
            pred_sb = pp.tile([128, NT, BL], F32, tag="pred")
            ident = pp.tile([128, 128], DT, tag="ident")
            ones = pp.tile([1, 128], DT, tag="ones")

            masks.make_identity(nc, ident[:])
            nc.gpsimd.memset(ones[:], 1.0)

            # weights
            W = {}
            for name, d in dram.items():
                if name[0] in "gc" and name[:2] in ("g0", "c0", "g1", "c1"):
                    W[name] = wp.tile(list(d.shape), DT, tag=name, name=name)
                    nc.sync.dma_start(W[name][:], d[:, :])
            Wp_sb = wp.tile([U, 1], DT, tag="Wp")
            nc.sync.dma_start(Wp_sb[:], Wp_d[:, :])
            bp_sb = wp.tile([1, BL], DT, tag="bp")
            nc.sync.dma_start(bp_sb[:], bp_d[:, :])

            # input loads
            nc.sync.dma_start(x0g0[:], dram_tiled(dram["x0g0"]))
            nc.sync.dma_start(x0c0[:], dram_tiled(dram["x0c0"]))
            nc.sync.dma_start(h0[:], dram_tiled(dram["h0"]))
            nc.sync.dma_start(h1[:], dram_tiled(dram["h1"]))

            # ---- helpers ----
            def stream_block(ST_d, mt):
                st = stp.tile([128, NT, 128], DT, tag="st")
                src = ST_d.rearrange("(t p) m -> p t m", p=128)
                nc.sync.dma_start(st[:], src[:, :, mt * 128:(mt + 1) * 128])
                return st

            class Sliced:
                def __init__(self, t, cols):
                    self.t, self.cols = t, cols

                def __getitem__(self, idx):
                    p, nt, _ = idx
                    return self.t[p, nt, 0:self.cols]

            def s_app_psum(st, groups, width):
                """One m-tile of S@x accumulated over NT n-tiles.

                groups: list of sbuf tensors whose columns concatenate
                (group-major) to x's feature columns.  Returns 2D psum tile
                [128, BL*width] laid out [group][batch][w]."""
                ps = pAux.tile([128, BL * width], F32, tag="pAux", name="ps")
                gw = width // len(groups)
                last_g = len(groups) - 1
                for nt in range(NT):
                    for gi, g_sb in enumerate(groups):
                        dst = (ps[:] if len(groups) == 1 else
                               ps[:, gi * BL * gw:(gi + 1) * BL * gw])
                        nc.tensor.matmul(
                            dst, st[:, nt, :], g_sb[:, nt, :],
                            start=(nt == 0 and gi == 0),
                            stop=(nt == NT - 1 and gi == last_g),
                        )
                return ps

            def transpose_tile(src_ap, w):
                """[128, w] sbuf -> [w, 128] sbuf (bf16), via PE."""
                pt = pT.tile([128, 128], DT, tag="pT", name="pt")
                nc.tensor.transpose(pt[:w, :], src_ap, ident[:])
                tt = trp.tile([128, 128], DT, tag="tt", name="tt")
                nc.vector.tensor_copy(tt[:w, :], pt[:w, :])
                return tt

            def proj_slices(g, wname, tensor, mt, D):
                """Yield (src_ap, weight_tile, w) feature-group projection
                pieces of `tensor` (layout: group-major if D==D1)."""
                if D == D0:
                    for b in range(BL):
                        yield (tensor[:, mt, b * D0:(b + 1) * D0],
                               W[f"{g}_{wname}"], D0)
                else:
                    for gi, suf in enumerate("ab"):
                        for b in range(BL):
                            off = gi * BL * U + b * U
                            yield (tensor[:, mt, off:off + U],
                                   W[f"{g}_{wname}{suf}"], U)

            def consumer_gates(mt, ops, h_t, rh_dst_ap, u_dst_ap):
                # ops: psum [128, BL*2U]
                ops3 = ops.rearrange("p (b w) -> p b w", w=2 * U)
                r = smp.tile([128, BL, U], DT, tag="r", name="r")
                nc.scalar.activation(r[:], ops3[:, :, 0:U], AF.Sigmoid)
                nc.scalar.activation(u_dst_ap, ops3[:, :, U:2 * U], AF.Sigmoid)
                nc.vector.tensor_mul(rh_dst_ap, r[:], h_t)

            def consumer_cand(mt, ops, h_t, u_t, hp_dst_ap, hout_view):
                # ops: psum [128, BL*U];  h' = c + u*(h-c)
                c = smp.tile([128, BL * U], DT, tag="c", name="c")
                nc.scalar.activation(c[:], ops[:], AF.Tanh)
                t = smp.tile([128, BL * U], DT, tag="t", name="t")
                nc.vector.tensor_sub(t[:], h_t, c[:])
                nc.vector.tensor_mul(t[:], t[:], u_t)
                nc.vector.tensor_add(hp_dst_ap, c[:], t[:])
                nc.sync.dma_start(hout_view, hp_dst_ap)

            def gconv_pathA(g, x0_groups, x0_proj, x1_tag, D, OW, consume):
                """Full gconv, S#2 applied to x1 (width D), all projections at
                OUT time.  x0_groups: sbuf tensors concatenating to x0.
                x0_proj: list of (tensor, weight_suffix_or_None) per group for
                the identity-weight projection."""
                x1 = {}
                for s, ST_d in (("f", STf_d), ("b", STb_d)):
                    x1[s] = pp.tile([128, NT, BL * D1], DT,
                                    tag=f"x1{s}", name=f"x1{s}")
                    for mt in range(NT):
                        st = stream_block(ST_d, mt)
                        ps = s_app_psum(st, x0_groups, D)
                        nc.vector.tensor_copy(x1[s][:, mt, 0:BL * D], ps[:])
                for mt in range(NT):
                    stf = stream_block(STf_d, mt)
                    stb = stream_block(STb_d, mt)
                    y2 = {}
                    for s, st in (("f", stf), ("b", stb)):
                        ps = s_app_psum(st, [Sliced(x1[s], BL * D)], D)
                        y2[s] = smp.tile([128, BL * D], DT, tag="y2", name="y2")
                        nc.vector.tensor_copy(y2[s][:], ps[:])
                    ops = pOut.tile([128, BL * OW], F32, tag="pOut", name="ops")
                    nc.tensor.matmul(ops[:], ones[:], W[f"{g}_bias"][:],
                                     start=True, stop=False)
                    projs = []
                    # x0 identity projections
                    if D == D0:
                        for b in range(BL):
                            projs.append((x0_groups[0][:, mt, b * D0:(b + 1) * D0],
                                          W[f"{g}_wIa"] if False else W[f"{g}_wI"],
                                          D0, b))
                    else:
                        for gi, (src, suf) in enumerate(x0_proj):
                            for b in range(BL):
                                projs.append((src[:, mt, b * U:(b + 1) * U],
                                              W[f"{g}_wI{suf}"], U, b))
                    # x1 / y2 projections
                    for wname, tensors in (("w1", x1), ("w2", y2)):
                        for s in ("f", "b"):
                            tsr = tensors[s]
                            if D == D0:
                                for b in range(BL):
                                    src = (tsr[:, mt, b * D0:(b + 1) * D0]
                                           if wname == "w1" else
                                           tsr[:, b * D0:(b + 1) * D0])
                                    projs.append((src, W[f"{g}_{wname}{s}"],
                                                  D0, b))
                            else:
                                for gi, suf in enumerate("ab"):
                                    for b in range(BL):
                                        off = gi * BL * U + b * U
                                        src = (tsr[:, mt, off:off + U]
                                               if wname == "w1" else
                                               tsr[:, off:off + U])
                                        projs.append((src,
                                                      W[f"{g}_{wname}{s}{suf}"],
                                                      U, b))
                    for i, (src_ap, Wt, w, b) in enumerate(projs):
                        tt = transpose_tile(src_ap, w)
                        nc.tensor.matmul(ops[:, b * OW:(b + 1) * OW],
                                         tt[:w, :], Wt[:, :],
                                         start=False, stop=(i == len(projs) - 1))
                    consume(mt, ops)

            # ================= layer 0, gates =================
            def g0_consume(mt, ops):
                rh_dst = x0c0[:, mt, :].rearrange("p (b d) -> p b d", d=D0)[:, :, 1:]
                consumer_gates(mt, ops,
                               h0[:, mt, :].rearrange("p (b w) -> p b w", w=U),
                               rh_dst,
                               u_sb[:, mt, :].rearrange("p (b w) -> p b w", w=U))

            gconv_pathA("g0", [x0g0], None, "x1n", D0, 2 * U, g0_consume)

            # ================= layer 0, candidate =================
            def c0_consume(mt, ops):
                hview = dram_tiled(h0o_d)[:, mt, :]
                consumer_cand(mt, ops, h0[:, mt, :], u_sb[:, mt, :],
                              h0p[:, mt, :], hview)

            gconv_pathA("c0", [x0c0], None, "x1n", D0, U, c0_consume)

            # ================= layer 1, gates =================
            rh1 = pp.tile([128, NT, BL * U], DT, tag="h0", name="rh1")

            def g1_consume(mt, ops):
                consumer_gates(mt, ops,
                               h1[:, mt, :].rearrange("p (b w) -> p b w", w=U),
                               rh1[:, mt, :].rearrange("p (b w) -> p b w", w=U),
                               u_sb[:, mt, :].rearrange("p (b w) -> p b w", w=U))

            gconv_pathA("g1", [h0p, h1], [(h0p, "a"), (h1, "b")],
                        "x1w", D1, 2 * U, g1_consume)

            # ========== layer 1, candidate (projection-first) ==========
            proj1 = pp.tile([128, NT, BL * U], F32, tag="x1b", name="proj1")
            z_f = pp.tile([128, NT, BL * U], DT, tag="x0g0", name="z_f")
            z_b = pp.tile([128, NT, BL * U], DT, tag="x0c0", name="z_b")
            c1_groups = [h0p, rh1]
            for s, ST_d, z_sb in (("f", STf_d, z_f), ("b", STb_d, z_b)):
                x1 = pp.tile([128, NT, BL * D1], DT, tag="x1f", name="x1c1")
                for mt in range(NT):
                    st = stream_block(ST_d, mt)
                    ps = s_app_psum(st, c1_groups, D1)
                    nc.vector.tensor_copy(x1[:, mt, :], ps[:])
                # z pass + first-order projection accumulation
                for mt in range(NT):
                    for b in range(BL):
                        tta = transpose_tile(x1[:, mt, b * U:(b + 1) * U], U)
                        ttb = transpose_tile(x1[:, mt, BL * U + b * U:
                                                BL * U + (b + 1) * U], U)
                        psz = pS.tile([128, U], F32, tag="pS", name="psz")
                        nc.tensor.matmul(psz[:], tta[:U, :], W[f"c1_w2{s}a"][:],
                                         start=True, stop=False)
                        nc.tensor.matmul(psz[:], ttb[:U, :], W[f"c1_w2{s}b"][:],
                                         start=False, stop=True)
                        nc.vector.tensor_copy(z_sb[:, mt, b * U:(b + 1) * U],
                                              psz[:])
                        psp = pS.tile([128, U], F32, tag="pS", name="psp")
                        if s == "f":
                            nc.tensor.matmul(psp[:], tta[:U, :], W["c1_w1fa"][:],
                                             start=True, stop=False)
                            nc.tensor.matmul(psp[:], ttb[:U, :], W["c1_w1fb"][:],
                                             start=False, stop=False)
                            t0 = transpose_tile(h0p[:, mt, b * U:(b + 1) * U], U)
                            nc.tensor.matmul(psp[:], t0[:U, :], W["c1_wIa"][:],
                                             start=False, stop=False)
                            t1 = transpose_tile(rh1[:, mt, b * U:(b + 1) * U], U)
                            nc.tensor.matmul(psp[:], t1[:U, :], W["c1_wIb"][:],
                                             start=False, stop=True)
                            nc.vector.tensor_copy(proj1[:, mt, b * U:(b + 1) * U],
                                                  psp[:])
                        else:
                            nc.tensor.matmul(psp[:], tta[:U, :], W["c1_w1ba"][:],
                                             start=True, stop=False)
                            nc.tensor.matmul(psp[:], ttb[:U, :], W["c1_w1bb"][:],
                                             start=False, stop=True)
                            nc.vector.tensor_add(proj1[:, mt, b * U:(b + 1) * U],
                                                 proj1[:, mt, b * U:(b + 1) * U],
                                                 psp[:])

            for mt in range(NT):
                stf = stream_block(STf_d, mt)
                stb = stream_block(STb_d, mt)
                ops = pOut.tile([128, BL * U], F32, tag="pOut", name="ops")
                nc.tensor.matmul(ops[:], ones[:], W["c1_bias"][:],
                                 start=True, stop=False)
                for si, (st, z_sb) in enumerate(((stf, z_f), (stb, z_b))):
                    for nt in range(NT):
                        nc.tensor.matmul(ops[:], st[:, nt, :], z_sb[:, nt, :],
                                         start=False,
                                         stop=(si == 1 and nt == NT - 1))
                nc.vector.tensor_add(ops[:], ops[:], proj1[:, mt, :])
                # consumer: c, h1', pred
                c = smp.tile([128, BL * U], DT, tag="c", name="c")
                nc.scalar.activation(c[:], ops[:], AF.Tanh)
                t = smp.tile([128, BL * U], DT, tag="t", name="t")
                nc.vector.tensor_sub(t[:], h1[:, mt, :], c[:])
                nc.vector.tensor_mul(t[:], t[:], u_sb[:, mt, :])
                hp = smp.tile([128, BL * U], DT, tag="hp", name="hp")
                nc.vector.tensor_add(hp[:], c[:], t[:])
                nc.sync.dma_start(dram_tiled(h1o_d)[:, mt, :], hp[:])
                psp = pS.tile([128, BL], F32, tag="pS", name="pspred")
                nc.tensor.matmul(psp[:], ones[:], bp_sb[:], start=True, stop=False)
                for b in range(BL):
                    tt = transpose_tile(hp[:, b * U:(b + 1) * U], U)
                    nc.tensor.matmul(psp[:, b:b + 1], tt[:U, :], Wp_sb[:],
                                     start=False, stop=(b == BL - 1))
                nc.vector.tensor_copy(pred_sb[:, mt, :], psp[:])

            nc.sync.dma_start(dram_tiled(pred_d)[:, :, :], pred_sb[:])

    nc.compile()
    return nc


# ---------------------------------------------------------------------------
# host side
# ---------------------------------------------------------------------------

def _fold_weights(Wg, bg, Wc, bc, D):
    out = {}
    for pre, Wm, bm in (("g", Wg, bg), ("c", Wc, bc)):
        Wm = np.asarray(Wm, dtype=np.float32)
        WI, W1f, W2f, W1b, W2b = (Wm[i * D:(i + 1) * D] for i in range(5))
        folded = {"wI": WI - W2f - W2b, "w1f": W1f, "w1b": W1b,
                  "w2f": 2.0 * W2f, "w2b": 2.0 * W2b}
        for k, v in folded.items():
            if D == D0:
                out[f"{pre}_{k}"] = bf(v)
            else:
                out[f"{pre}_{k}a"] = bf(v[0:U])
                out[f"{pre}_{k}b"] = bf(v[U:2 * U])
        out[f"{pre}_bias"] = bf(np.tile(np.asarray(bm, np.float32), BL)[None, :])
    return out


def make_in_maps(inputs, n_nodes):
    """Build per-core input maps (host sharding + layout)."""
    x = np.asarray(inputs["inputs"], np.float32)          # (B, N, 1)
    hid = np.asarray(inputs["hidden_state"], np.float32)  # (2, B, N, U)
    STf = bf(np.asarray(inputs["S_fwd"], np.float32).T)
    STb = bf(np.asarray(inputs["S_bwd"], np.float32).T)

    w0 = _fold_weights(inputs["Wg0"], inputs["bg0"], inputs["Wc0"],
                       inputs["bc0"], D0)
    w1 = _fold_weights(inputs["Wg1"], inputs["bg1"], inputs["Wc1"],
                       inputs["bc1"], D1)
    Wp = bf(inputs["Wp"])
    bp = bf(np.tile(np.asarray(inputs["bp"], np.float32), BL)[None, :])

    suffixes0 = [w for w in WNAMES] + ["bias"]
    suffixes1 = [w + s for w in WNAMES for s in "ab"] + ["bias"]

    in_maps = []
    for c in range(NCORES):
        bs = slice(c * BL, (c + 1) * BL)
        xs = x[bs]                    # (BL, N, 1)
        h0s = hid[0][bs]              # (BL, N, U)
        h1s = hid[1][bs]
        x0g0 = np.concatenate([xs, h0s], axis=-1)         # (BL, N, D0)
        x0g0 = bf(x0g0.transpose(1, 0, 2).reshape(n_nodes, BL * D0))
        x0c0 = np.zeros((BL, n_nodes, D0), np.float32)
        x0c0[:, :, 0] = xs[:, :, 0]
        x0c0 = bf(x0c0.transpose(1, 0, 2).reshape(n_nodes, BL * D0))
        m = {
            "STf": STf, "STb": STb,
            "x0g0": x0g0, "x0c0": x0c0,
            "h0": bf(h0s.transpose(1, 0, 2).reshape(n_nodes, BL * U)),
            "h1": bf(h1s.transpose(1, 0, 2).reshape(n_nodes, BL * U)),
            "Wp": Wp, "bp": bp,
        }
        for pre, wd, sufs in (("g0", w0, suffixes0), ("c0", w0, suffixes0),
                              ("g1", w1, suffixes1), ("c1", w1, suffixes1)):
            src = pre[0]
            for wn in sufs:
                m[f"{pre}_{wn}"] = wd[f"{src}_{wn}"]
        in_maps.append(m)
    return in_maps


def gather_outputs(results, n_nodes):
    preds, h0s, h1s = [], [], []
    for r in results:
        pred = np.asarray(r["pred"], np.float32)          # (N, BL)
        h0o = np.asarray(r["h0o"]).astype(np.float32)     # (N, BL*U)
        h1o = np.asarray(r["h1o"]).astype(np.float32)
        preds.append(pred.T[:, :, None])                  # (BL, N, 1)
        h0s.append(h0o.reshape(n_nodes, BL, U).transpose(1, 0, 2))
        h1s.append(h1o.reshape(n_nodes, BL, U).transpose(1, 0, 2))
    predict = np.concatenate(preds, axis=0)
    new_h = np.stack([np.concatenate(h0s, 0), np.concatenate(h1s, 0)])
    return predict, new_h


_CACHE = {}


def kernel(**inputs):
    n_nodes = np.asarray(inputs["inputs"]).shape[1]
    if n_nodes not in _CACHE:
        _CACHE[n_nodes] = build_program(n_nodes)
    nc = _CACHE[n_nodes]
    in_maps = make_in_maps(inputs, n_nodes)
    from concourse import bass_utils
    res = bass_utils.run_bass_kernel_spmd(nc, in_maps,
                                          core_ids=list(range(NCORES)))
    return gather_outputs(res.results, n_nodes)


# revision 12
# speedup vs baseline: 1.3349x; 1.3349x over previous
"""DCRNN decoder (2-layer DCGRU, K=2 diffusion, 2 supports) on 8 TRN2 NeuronCores.

Strategy: data-parallel over batch (B=32 -> 4 per core), supports streamed from
HBM as bf16 S^T column-blocks, all matmul work in bf16 with fp32 PSUM
accumulation.  Chebyshev "2*S@x1 - x0" terms and biases are folded into
host-side weight adjustments so the whole gconv is pure matmul accumulation:

  gconv(x0) = x0@(WI - W2f - W2b) + sum_s [ (Ss@x0)@W1s + (Ss@(Ss@x0))@(2*W2s) ] + b

S-applications run with the S^T tile stationary and activations as the moving
operand (node-major PSUM output).  Projections contract the feature dim, so
activation tiles are PE-transposed per (m-tile, batch, feature-group) on the
fly.  For D=128 gconvs, features are handled as two 64-wide groups and the
weights are split into 64-row halves host-side (matmul operands must share a
base partition).  For the layer-1 candidate gconv (D=128 -> out=64) the
projection-first form halves the second diffusion matmul width.
"""

import sys

import numpy as np

sys.path.insert(0, "/opt/trn_rl_repo")

import ml_dtypes

import concourse.bacc as bacc
import concourse.mybir as mybir
import concourse.tile as tile
from concourse import masks

bf16 = ml_dtypes.bfloat16
DT = mybir.dt.bfloat16
F32 = mybir.dt.float32
AF = mybir.ActivationFunctionType

B = 32
U = 64
NCORES = 8
BL = B // NCORES          # batch per core
D0 = U + 1                # layer-0 gconv input width
D1 = 2 * U                # layer-1 gconv input width
WNAMES = ("wI", "w1f", "w1b", "w2f", "w2b")
GCONVS = {
    "g0": dict(D=D0, OW=2 * U),
    "c0": dict(D=D0, OW=U),
    "g1": dict(D=D1, OW=2 * U),
    "c1": dict(D=D1, OW=U),
}


def bf(x):
    return np.ascontiguousarray(np.asarray(x, dtype=np.float32)).astype(bf16)


# ---------------------------------------------------------------------------
# device program
# ---------------------------------------------------------------------------

def build_program(n_nodes):
    NT = n_nodes // 128
    nc = bacc.Bacc("TRN2", target_bir_lowering=False, debug=False,
                   num_devices=NCORES)

    dram = {}

    def din(name, shape, dt=DT):
        dram[name] = nc.dram_tensor(name, shape, dt, kind="ExternalInput")
        return dram[name]

    def dout(name, shape, dt=DT):
        dram[name] = nc.dram_tensor(name, shape, dt, kind="ExternalOutput")
        return dram[name]

    STf_d = din("STf", (n_nodes, n_nodes))
    STb_d = din("STb", (n_nodes, n_nodes))
    din("x0g0", (n_nodes, BL * D0))
    din("x0c0", (n_nodes, BL * D0))
    din("h0", (n_nodes, BL * U))
    din("h1", (n_nodes, BL * U))
    for g, cfg in GCONVS.items():
        if cfg["D"] == D0:
            for w in WNAMES:
                din(f"{g}_{w}", (D0, cfg["OW"]))
        else:
            for w in ("w1f", "w1b", "w2f", "w2b"):
                din(f"{g}_{w}", (D1, cfg["OW"]))
            din(f"{g}_wIa", (U, cfg["OW"]))
            din(f"{g}_wIb", (U, cfg["OW"]))
        din(f"{g}_bias", (1, BL * cfg["OW"]))
    Wp2_d = din("Wp2", (2 * U, 1))
    bp_d = din("bp", (1, BL))

    pred_d = dout("pred", (n_nodes, BL), F32)
    h0o_d = dout("h0o", (n_nodes, BL * U))
    h1o_d = dout("h1o", (n_nodes, BL * U))

    def dram_tiled(d):
        # [N, C] dram -> [128, NT, C] node-major tile view
        return d.rearrange("(t p) c -> p t c", p=128)

    with tile.TileContext(nc) as tc:
        with (
            tc.tile_pool(name="persist", bufs=1) as pp,
            tc.tile_pool(name="wpool", bufs=1) as wp,
            tc.tile_pool(name="stream", bufs=3) as stp,
            tc.tile_pool(name="trans", bufs=6) as trp,
            tc.tile_pool(name="small", bufs=3) as smp,
            tc.tile_pool(name="pOut", bufs=2, space="PSUM") as pOut,
            tc.tile_pool(name="pAux", bufs=2, space="PSUM") as pAux,
            tc.tile_pool(name="pT", bufs=2, space="PSUM") as pT,
            tc.tile_pool(name="pS", bufs=2, space="PSUM") as pS,
        ):
            # ---- persistent SBUF tensors ----
            x0g0 = pp.tile([128, NT, BL * D0], DT, tag="x0g0")
            x0c0 = pp.tile([128, NT, BL * D0], DT, tag="x0c0")
            h0 = pp.tile([128, NT, BL * U], DT, tag="h0")
            h1 = pp.tile([128, NT, BL * U], DT, tag="h1")
            h0p = pp.tile([128, NT, BL * U], DT, tag="h0p")
            u_sb = pp.tile([128, NT, BL * U], DT, tag="u")
            pred_sb = pp.tile([128, NT, BL], F32, tag="pred")
            ident = pp.tile([128, 128], DT, tag="ident")
            ones = pp.tile([1, 128], DT, tag="ones")

            masks.make_identity(nc, ident[:])
            nc.gpsimd.memset(ones[:], 1.0)

            # weights
            W = {}
            for name, d in dram.items():
                if name[0] in "gc" and name[:2] in ("g0", "c0", "g1", "c1"):
                    W[name] = wp.tile(list(d.shape), DT, tag=name, name=name)
                    nc.sync.dma_start(W[name][:], d[:, :])
            Wp2_sb = wp.tile([2 * U, 1], DT, tag="Wp2")
            nc.sync.dma_start(Wp2_sb[:], Wp2_d[:, :])
            bp_sb = wp.tile([1, BL], DT, tag="bp")
            nc.sync.dma_start(bp_sb[:], bp_d[:, :])

            # input loads
            nc.sync.dma_start(x0g0[:], dram_tiled(dram["x0g0"]))
            nc.sync.dma_start(x0c0[:], dram_tiled(dram["x0c0"]))
            nc.sync.dma_start(h0[:], dram_tiled(dram["h0"]))
            nc.sync.dma_start(h1[:], dram_tiled(dram["h1"]))

            # ---- helpers ----
            def stream_block(ST_d, mt):
                # host supplies ST permuted so block mt is one contiguous
                # [128, NT*128] row-chunk: STperm[mt*128+p, t*128+j]
                st = stp.tile([128, NT, 128], DT, tag="st")
                src = ST_d.rearrange("(mt p) c -> mt p c", p=128)
                nc.sync.dma_start(st[:], src[mt, :, :])
                return st

            class Sliced:
                def __init__(self, t, cols):
                    self.t, self.cols = t, cols

                def __getitem__(self, idx):
                    p, nt, _ = idx
                    return self.t[p, nt, 0:self.cols]

            def s_app_psum(st, groups, width):
                """One m-tile of S@x accumulated over NT n-tiles.

                groups: list of sbuf tensors whose columns concatenate
                (group-major) to x's feature columns.  Returns 2D psum tile
                [128, BL*width] laid out [group][batch][w]."""
                ps = pAux.tile([128, BL * width], F32, tag="pAux", name="ps")
                gw = width // len(groups)
                last_g = len(groups) - 1
                for nt in range(NT):
                    for gi, g_sb in enumerate(groups):
                        dst = (ps[:] if len(groups) == 1 else
                               ps[:, gi * BL * gw:(gi + 1) * BL * gw])
                        nc.tensor.matmul(
                            dst, st[:, nt, :], g_sb[:, nt, :],
                            start=(nt == 0 and gi == 0),
                            stop=(nt == NT - 1 and gi == last_g),
                        )
                return ps

            def transpose_tile(src_ap, w):
                """[128, w] sbuf -> [w, 128] sbuf (bf16), via PE."""
                pt = pT.tile([128, 128], DT, tag="pT", name="pt")
                nc.tensor.transpose(pt[:w, :], src_ap, ident[:])
                tt = trp.tile([128, 128], DT, tag="tt", name="tt")
                nc.vector.tensor_copy(tt[:w, :], pt[:w, :])
                return tt

            def proj_slices(g, wname, tensor, mt, D):
                """Yield (src_ap, weight_tile, w) feature-group projection
                pieces of `tensor` (layout: group-major if D==D1)."""
                if D == D0:
                    for b in range(BL):
                        yield (tensor[:, mt, b * D0:(b + 1) * D0],
                               W[f"{g}_{wname}"], D0)
                else:
                    for gi, suf in enumerate("ab"):
                        for b in range(BL):
                            off = gi * BL * U + b * U
                            yield (tensor[:, mt, off:off + U],
                                   W[f"{g}_{wname}{suf}"], U)

            def consumer_gates(mt, ops, h_t, rh_dst_ap, u_dst_ap):
                # ops: psum [128, BL*2U]
                ops3 = ops.rearrange("p (b w) -> p b w", w=2 * U)
                r = smp.tile([128, BL, U], DT, tag="r", name="r")
                nc.scalar.activation(r[:], ops3[:, :, 0:U], AF.Sigmoid)
                nc.scalar.activation(u_dst_ap, ops3[:, :, U:2 * U], AF.Sigmoid)
                nc.vector.tensor_mul(rh_dst_ap, r[:], h_t)

            def consumer_cand(mt, ops, h_t, u_t, hp_dst_ap, hout_view):
                # ops: psum [128, BL*U];  h' = c + u*(h-c)
                c = smp.tile([128, BL * U], DT, tag="c", name="c")
                nc.scalar.activation(c[:], ops[:], AF.Tanh)
                t = smp.tile([128, BL * U], DT, tag="t", name="t")
                nc.vector.tensor_sub(t[:], h_t, c[:])
                nc.vector.tensor_mul(t[:], t[:], u_t)
                nc.vector.tensor_add(hp_dst_ap, c[:], t[:])
                nc.sync.dma_start(hout_view, hp_dst_ap)

            def gconv_pathA(g, x0_groups, x0_proj, x1_tag, D, OW, consume):
                """Full gconv, S#2 applied to x1 (width D), all projections at
                OUT time.  x0_groups: sbuf tensors concatenating to x0.
                x0_proj: list of (tensor, weight_suffix_or_None) per group for
                the identity-weight projection."""
                x1 = {}
                for s, ST_d in (("f", STf_d), ("b", STb_d)):
                    x1[s] = pp.tile([128, NT, BL * D1], DT,
                                    tag=f"x1{s}", name=f"x1{s}")
                    for mt in range(NT):
                        st = stream_block(ST_d, mt)
                        ps = s_app_psum(st, x0_groups, D)
                        if len(x0_groups) > 1:
                            d3 = x1[s][:, mt, 0:BL * D].rearrange(
                                "p (b gw) -> p b gw", gw=D)
                            for gi in range(len(x0_groups)):
                                nc.vector.tensor_copy(
                                    d3[:, :, gi * U:(gi + 1) * U],
                                    ps[:, gi * BL * U:(gi + 1) * BL * U])
                        else:
                            nc.vector.tensor_copy(x1[s][:, mt, 0:BL * D], ps[:])
                for mt in range(NT):
                    stf = stream_block(STf_d, mt)
                    stb = stream_block(STb_d, mt)
                    y2 = {}
                    for s, st in (("f", stf), ("b", stb)):
                        ps = s_app_psum(st, [Sliced(x1[s], BL * D)], D)
                        y2[s] = smp.tile([128, BL * D], DT, tag="y2", name="y2")
                        nc.vector.tensor_copy(y2[s][:], ps[:])
                    ops = pOut.tile([128, BL * OW], F32, tag="pOut", name="ops")
                    nc.tensor.matmul(ops[:], ones[:], W[f"{g}_bias"][:],
                                     start=True, stop=False)
                    projs = []
                    # x0 identity projections
                    if D == D0:
                        for b in range(BL):
                            projs.append((x0_groups[0][:, mt, b * D0:(b + 1) * D0],
                                          W[f"{g}_wIa"] if False else W[f"{g}_wI"],
                                          D0, b))
                    else:
                        for gi, (src, suf) in enumerate(x0_proj):
                            for b in range(BL):
                                projs.append((src[:, mt, b * U:(b + 1) * U],
                                              W[f"{g}_wI{suf}"], U, b))
                    # x1 / y2 projections; for D1 the two 64-wide feature
                    # groups pack into one 128-wide transpose + matmul
                    for wname, tensors in (("w1", x1), ("w2", y2)):
                        for s in ("f", "b"):
                            tsr = tensors[s]
                            if D == D0:
                                for b in range(BL):
                                    src = (tsr[:, mt, b * D0:(b + 1) * D0]
                                           if wname == "w1" else
                                           tsr[:, b * D0:(b + 1) * D0])
                                    projs.append((src, W[f"{g}_{wname}{s}"],
                                                  D0, b))
                            else:
                                flat = (tsr[:, mt, :] if wname == "w1"
                                        else tsr[:])
                                for b in range(BL):
                                    projs.append((flat[:, b * D1:(b + 1) * D1],
                                                  W[f"{g}_{wname}{s}"],
                                                  D1, b))
                    for i, (src_ap, Wt, w, b) in enumerate(projs):
                        tt = transpose_tile(src_ap, w)
                        nc.tensor.matmul(ops[:, b * OW:(b + 1) * OW],
                                         tt[:w, :], Wt[:, :],
                                         start=False, stop=(i == len(projs) - 1))
                    consume(mt, ops)

            # ================= layer 0, gates =================
            def g0_consume(mt, ops):
                rh_dst = x0c0[:, mt, :].rearrange("p (b d) -> p b d", d=D0)[:, :, 1:]
                consumer_gates(mt, ops,
                               h0[:, mt, :].rearrange("p (b w) -> p b w", w=U),
                               rh_dst,
                               u_sb[:, mt, :].rearrange("p (b w) -> p b w", w=U))

            gconv_pathA("g0", [x0g0], None, "x1n", D0, 2 * U, g0_consume)

            # ================= layer 0, candidate =================
            def c0_consume(mt, ops):
                hview = dram_tiled(h0o_d)[:, mt, :]
                consumer_cand(mt, ops, h0[:, mt, :], u_sb[:, mt, :],
                              h0p[:, mt, :], hview)

            gconv_pathA("c0", [x0c0], None, "x1n", D0, U, c0_consume)

            # ================= layer 1, gates =================
            rh1 = pp.tile([128, NT, BL * U], DT, tag="h0", name="rh1")

            def g1_consume(mt, ops):
                consumer_gates(mt, ops,
                               h1[:, mt, :].rearrange("p (b w) -> p b w", w=U),
                               rh1[:, mt, :].rearrange("p (b w) -> p b w", w=U),
                               u_sb[:, mt, :].rearrange("p (b w) -> p b w", w=U))

            gconv_pathA("g1", [h0p, h1], [(h0p, "a"), (h1, "b")],
                        "x1w", D1, 2 * U, g1_consume)

            # ========== layer 1, candidate (projection-first) ==========
            proj1 = pp.tile([128, NT, BL * U], F32, tag="x1b", name="proj1")
            z_f = pp.tile([128, NT, BL * U], DT, tag="x0g0", name="z_f")
            z_b = pp.tile([128, NT, BL * U], DT, tag="x0c0", name="z_b")
            c1_groups = [h0p, rh1]
            for s, ST_d, z_sb in (("f", STf_d, z_f), ("b", STb_d, z_b)):
                x1 = pp.tile([128, NT, BL * D1], DT, tag="x1f", name="x1c1")
                for mt in range(NT):
                    st = stream_block(ST_d, mt)
                    ps = s_app_psum(st, c1_groups, D1)
                    d3 = x1[:, mt, :].rearrange("p (b gw) -> p b gw", gw=D1)
                    for gi in range(2):
                        nc.vector.tensor_copy(
                            d3[:, :, gi * U:(gi + 1) * U],
                            ps[:, gi * BL * U:(gi + 1) * BL * U])
                # z pass + first-order projection accumulation
                for mt in range(NT):
                    for b in range(BL):
                        tt = transpose_tile(x1[:, mt, b * D1:(b + 1) * D1], D1)
                        psz = pS.tile([128, U], F32, tag="pS", name="psz")
                        nc.tensor.matmul(psz[:], tt[:], W[f"c1_w2{s}"][:],
                                         start=True, stop=True)
                        nc.vector.tensor_copy(z_sb[:, mt, b * U:(b + 1) * U],
                                              psz[:])
                        psp = pS.tile([128, U], F32, tag="pS", name="psp")
                        if s == "f":
                            nc.tensor.matmul(psp[:], tt[:], W["c1_w1f"][:],
                                             start=True, stop=False)
                            t0 = transpose_tile(h0p[:, mt, b * U:(b + 1) * U], U)
                            nc.tensor.matmul(psp[:], t0[:U, :], W["c1_wIa"][:],
                                             start=False, stop=False)
                            t1 = transpose_tile(rh1[:, mt, b * U:(b + 1) * U], U)
                            nc.tensor.matmul(psp[:], t1[:U, :], W["c1_wIb"][:],
                                             start=False, stop=True)
                            nc.vector.tensor_copy(proj1[:, mt, b * U:(b + 1) * U],
                                                  psp[:])
                        else:
                            nc.tensor.matmul(psp[:], tt[:], W["c1_w1b"][:],
                                             start=True, stop=True)
                            nc.vector.tensor_add(proj1[:, mt, b * U:(b + 1) * U],
                                                 proj1[:, mt, b * U:(b + 1) * U],
                                                 psp[:])

            for mt in range(NT):
                stf = stream_block(STf_d, mt)
                stb = stream_block(STb_d, mt)
                ops = pOut.tile([128, BL * U], F32, tag="pOut", name="ops")
                nc.tensor.matmul(ops[:], ones[:], W["c1_bias"][:],
                                 start=True, stop=False)
                for si, (st, z_sb) in enumerate(((stf, z_f), (stb, z_b))):
                    for nt in range(NT):
                        nc.tensor.matmul(ops[:], st[:, nt, :], z_sb[:, nt, :],
                                         start=False,
                                         stop=(si == 1 and nt == NT - 1))
                nc.vector.tensor_add(ops[:], ops[:], proj1[:, mt, :])
                # consumer: c, h1', pred
                c = smp.tile([128, BL * U], DT, tag="c", name="c")
                nc.scalar.activation(c[:], ops[:], AF.Tanh)
                t = smp.tile([128, BL * U], DT, tag="t", name="t")
                nc.vector.tensor_sub(t[:], h1[:, mt, :], c[:])
                nc.vector.tensor_mul(t[:], t[:], u_sb[:, mt, :])
                hp = smp.tile([128, BL * U], DT, tag="hp", name="hp")
                nc.vector.tensor_add(hp[:], c[:], t[:])
                nc.sync.dma_start(dram_tiled(h1o_d)[:, mt, :], hp[:])
                psp = pS.tile([128, BL], F32, tag="pS", name="pspred")
                nc.tensor.matmul(psp[:], ones[:], bp_sb[:], start=True, stop=False)
                for b in range(BL):
                    tt = transpose_tile(hp[:, b * U:(b + 1) * U], U)
                    nc.tensor.matmul(psp[:, b:b + 1], tt[:U, :],
                                     Wp2_sb[0:U, :],
                                     start=False, stop=(b == BL - 1))
                nc.vector.tensor_copy(pred_sb[:, mt, :], psp[:])

            nc.sync.dma_start(dram_tiled(pred_d)[:, :, :], pred_sb[:])

    nc.compile()
    return nc


# ---------------------------------------------------------------------------
# host side
# ---------------------------------------------------------------------------

def _fold_weights(Wg, bg, Wc, bc, D):
    out = {}
    for pre, Wm, bm in (("g", Wg, bg), ("c", Wc, bc)):
        Wm = np.asarray(Wm, dtype=np.float32)
        WI, W1f, W2f, W1b, W2b = (Wm[i * D:(i + 1) * D] for i in range(5))
        folded = {"wI": WI - W2f - W2b, "w1f": W1f, "w1b": W1b,
                  "w2f": 2.0 * W2f, "w2b": 2.0 * W2b}
        for k, v in folded.items():
            if D == D0:
                out[f"{pre}_{k}"] = bf(v)
            elif k == "wI":
                out[f"{pre}_wIa"] = bf(v[0:U])
                out[f"{pre}_wIb"] = bf(v[U:2 * U])
            else:
                out[f"{pre}_{k}"] = bf(v)
        out[f"{pre}_bias"] = bf(np.tile(np.asarray(bm, np.float32), BL)[None, :])
    return out


def make_in_maps(inputs, n_nodes):
    """Build per-core input maps (host sharding + layout)."""
    x = np.asarray(inputs["inputs"], np.float32)          # (B, N, 1)
    hid = np.asarray(inputs["hidden_state"], np.float32)  # (2, B, N, U)
    def perm_st(S):
        # ST[n, m] -> STperm[mt*128+p, t*128+j] = ST[t*128+p, mt*128+j]
        ST = np.ascontiguousarray(np.asarray(S, np.float32).T)
        NT = n_nodes // 128
        STp = ST.reshape(NT, 128, NT, 128).transpose(2, 1, 0, 3)
        return bf(STp.reshape(n_nodes, n_nodes))

    STf = perm_st(inputs["S_fwd"])
    STb = perm_st(inputs["S_bwd"])

    w0 = _fold_weights(inputs["Wg0"], inputs["bg0"], inputs["Wc0"],
                       inputs["bc0"], D0)
    w1 = _fold_weights(inputs["Wg1"], inputs["bg1"], inputs["Wc1"],
                       inputs["bc1"], D1)
    Wp2 = bf(np.concatenate([np.asarray(inputs["Wp"], np.float32)] * 2, axis=0))
    bp = bf(np.tile(np.asarray(inputs["bp"], np.float32), BL)[None, :])

    suffixes0 = [w for w in WNAMES] + ["bias"]
    suffixes1 = ["w1f", "w1b", "w2f", "w2b", "wIa", "wIb", "bias"]

    in_maps = []
    for c in range(NCORES):
        bs = slice(c * BL, (c + 1) * BL)
        xs = x[bs]                    # (BL, N, 1)
        h0s = hid[0][bs]              # (BL, N, U)
        h1s = hid[1][bs]
        x0g0 = np.concatenate([xs, h0s], axis=-1)         # (BL, N, D0)
        x0g0 = bf(x0g0.transpose(1, 0, 2).reshape(n_nodes, BL * D0))
        x0c0 = np.zeros((BL, n_nodes, D0), np.float32)
        x0c0[:, :, 0] = xs[:, :, 0]
        x0c0 = bf(x0c0.transpose(1, 0, 2).reshape(n_nodes, BL * D0))
        m = {
            "STf": STf, "STb": STb,
            "x0g0": x0g0, "x0c0": x0c0,
            "h0": bf(h0s.transpose(1, 0, 2).reshape(n_nodes, BL * U)),
            "h1": bf(h1s.transpose(1, 0, 2).reshape(n_nodes, BL * U)),
            "Wp2": Wp2, "bp": bp,
        }
        for pre, wd, sufs in (("g0", w0, suffixes0), ("c0", w0, suffixes0),
                              ("g1", w1, suffixes1), ("c1", w1, suffixes1)):
            src = pre[0]
            for wn in sufs:
                m[f"{pre}_{wn}"] = wd[f"{src}_{wn}"]
        in_maps.append(m)
    return in_maps


def gather_outputs(results, n_nodes):
    preds, h0s, h1s = [], [], []
    for r in results:
        pred = np.asarray(r["pred"], np.float32)          # (N, BL)
        h0o = np.asarray(r["h0o"]).astype(np.float32)     # (N, BL*U)
        h1o = np.asarray(r["h1o"]).astype(np.float32)
        preds.append(pred.T[:, :, None])                  # (BL, N, 1)
        h0s.append(h0o.reshape(n_nodes, BL, U).transpose(1, 0, 2))
        h1s.append(h1o.reshape(n_nodes, BL, U).transpose(1, 0, 2))
    predict = np.concatenate(preds, axis=0)
    new_h = np.stack([np.concatenate(h0s, 0), np.concatenate(h1s, 0)])
    return predict, new_h


_CACHE = {}


def kernel(**inputs):
    n_nodes = np.asarray(inputs["inputs"]).shape[1]
    if n_nodes not in _CACHE:
        _CACHE[n_nodes] = build_program(n_nodes)
    nc = _CACHE[n_nodes]
    in_maps = make_in_maps(inputs, n_nodes)
    from concourse import bass_utils
    res = bass_utils.run_bass_kernel_spmd(nc, in_maps,
                                          core_ids=list(range(NCORES)))
    return gather_outputs(res.results, n_nodes)
